# revision 10
# baseline (speedup 1.0000x reference)
# Trainium2 Bass kernel for nn_Decoder (GRU decoder step + attention + vocab softmax).
#
# Math note: the reference computes proj = enc @ W_attn.T + b_attn  (B*S*H*H flops)
# and then scores = einsum('bsh,bh->bs', proj, rnn_out).  Algebraically
#   scores[b,s] = enc[b,s] . (W_attn.T @ rnn_out[b]) + (b_attn . rnn_out[b])
# and the second term is constant per b, so it cancels inside softmax.  We
# therefore compute q[b] = rnn_out[b] @ W_attn (H*H flops per b) and contract
# enc with q directly — ~1000x fewer flops, numerically identical after softmax.
#
# Sharding (8 cores):
#   - GRU: output-dim sharded (each core computes a 128-wide h-slice of both
#     layers for all 32 batch rows); AllGather between layers.
#   - Attention: data-parallel over batch (4 b per core); enc shipped in both
#     natural [s,h] and transposed [h,s] layouts (host prep) so both the
#     h-contraction (scores) and s-contraction (context) run on the PE.
#   - Output projection: vocab-sharded (4000 vocab rows per core); the [32]
#     exp-sum partials are AllReduced for the exact global softmax.
import numpy as np

B, S, H, V = 32, 2048, 1024, 32000
NCORES = 8
BPC = B // NCORES      # 4 batch rows per core
VPC = V // NCORES      # 4000 vocab rows per core
HPC = H // NCORES      # 128 hidden slice per core

_cache = {}


def _vblocks(vpc):
    out = []
    o = 0
    while o < vpc:
        out.append((o, min(512, vpc - o)))
        o += 512
    return out


def build(S_=S, V_=V, tiled=True):
    # tiled=True runs the four per-batch-row M=1 matmul chains (scores /
    # context / attn-transpose) in separate PE column/row groups so the
    # hardware executes them concurrently (tile_position auto-derived from
    # the AP base partitions).
    import concourse.bacc as bacc
    import concourse.mybir as mybir
    import concourse.tile as tile

    f32 = mybir.dt.float32
    Alu = mybir.AluOpType
    Ax = mybir.AxisListType
    Act = mybir.ActivationFunctionType
    VPC_ = V_ // NCORES
    SCn = S_ // 128     # 128-wide s-chunks
    SBn = S_ // 512     # 512-wide s-blocks
    VB = _vblocks(VPC_)
    GROUPS = [list(range(NCORES))]

    nc = bacc.Bacc("TRN2", target_bir_lowering=False, debug=False,
                   num_devices=NCORES)

    di = lambda n, s: nc.dram_tensor(n, s, f32, kind="ExternalInput")
    do = lambda n, s: nc.dram_tensor(n, s, f32, kind="ExternalOutput")

    rnn_inT = di("rnn_inT", [2048, 32])
    h0prevT = di("h0prevT", [1024, 32])
    h1prevT = di("h1prevT", [1024, 32])
    h0sl_d = di("h0sl", [128, 32])
    h1sl_d = di("h1sl", [128, 32])
    wih0T = di("wih0T", [2048, 384])
    whh0T = di("whh0T", [1024, 384])
    wih1T = di("wih1T", [1024, 384])
    whh1T = di("whh1T", [1024, 384])
    bgru_d = di("bgru", [128, 8])
    wattn_d = di("wattn", [1024, 1024])
    bsel_d = di("bsel", [32, 4])
    eye_d = di("eye", [128, 128])
    encT_d = di("encT", [BPC, 1024, S_])
    encN_d = di("encN", [BPC, S_, 1024])
    woutT_d = di("woutT", [2048, VPC_])
    bout_d = di("bout", [1, VPC_])

    h0T_o = do("h0T_o", [128, 32])
    h1T_o = do("h1T_o", [128, 32])
    ctx_o = do("ctx_o", [BPC, 1024])
    attn_o = do("attn_o", [BPC, S_])
    probs_o = do("probs_o", [32, VPC_])

    with tile.TileContext(nc) as tc:
        with (
            tc.tile_pool(name="res", bufs=1) as res,
            tc.tile_pool(name="pbig", bufs=2) as pbig,
            tc.tile_pool(name="pw2", bufs=2) as pw2,
            tc.tile_pool(name="penc", bufs=6 if tiled else 3) as penc,
            tc.tile_pool(name="pencN", bufs=6 if tiled else 4) as pencN,
            tc.tile_pool(name="work", bufs=1) as work,
            tc.tile_pool(name="attnp", bufs=2) as attnp,
            tc.tile_pool(name="ps", bufs=8, space="PSUM") as ps,
            tc.tile_pool(name="dram", bufs=1, space="DRAM") as dram,
        ):
            # ---------------- resident small loads ----------------
            rit = res.tile([128, 16, 32], f32, tag="rit")
            nc.sync.dma_start(rit[:], rnn_inT.ap().rearrange("(k p) b -> p k b", p=128))
            h0pt = res.tile([128, 8, 32], f32, tag="h0pt")
            nc.sync.dma_start(h0pt[:], h0prevT.ap().rearrange("(k p) b -> p k b", p=128))
            h1pt = res.tile([128, 8, 32], f32, tag="h1pt")
            nc.sync.dma_start(h1pt[:], h1prevT.ap().rearrange("(k p) b -> p k b", p=128))
            h0sl = res.tile([128, 32], f32, tag="h0sl")
            nc.sync.dma_start(h0sl[:], h0sl_d.ap())
            h1sl = res.tile([128, 32], f32, tag="h1sl")
            nc.sync.dma_start(h1sl[:], h1sl_d.ap())
            bg = res.tile([128, 8], f32, tag="bg")
            nc.sync.dma_start(bg[:], bgru_d.ap())
            bselt = res.tile([32, 4], f32, tag="bselt")
            nc.sync.dma_start(bselt[:], bsel_d.ap())
            eyet = res.tile([128, 128], f32, tag="eyet")
            nc.sync.dma_start(eyet[:], eye_d.ap())
            boutt = res.tile([1, VPC_], f32, tag="boutt")
            nc.sync.dma_start(boutt[:], bout_d.ap())
            ones1 = res.tile([1, 1], f32, tag="ones1")
            nc.vector.memset(ones1[:], 1.0)
            ones32 = res.tile([1, 32], f32, tag="ones32")
            nc.vector.memset(ones32[:], 1.0)
            ones128 = res.tile([128, 1], f32, tag="ones128")
            nc.vector.memset(ones128[:], 1.0)

            # ---------------- GRU (output-sharded) ----------------
            def gru_layer(wi_d, wh_d, ki, rhs_i, rhs_h, hsl, bo, htag):
                wi = pbig.tile([128, ki, 384], f32, tag="big")
                nc.sync.dma_start(wi[:], wi_d.ap().rearrange("(k p) g -> p k g", p=128))
                wh = pbig.tile([128, 8, 384], f32, tag="big")
                nc.sync.dma_start(wh[:], wh_d.ap().rearrange("(k p) g -> p k g", p=128))
                gi = [ps.tile([128, 32], f32, tag="ps", name=f"gi{g}") for g in range(3)]
                for k in range(ki):
                    for g in range(3):
                        nc.tensor.matmul(gi[g][:], wi[:, k, g * 128:(g + 1) * 128],
                                         rhs_i[:, k, :],
                                         start=(k == 0), stop=(k == ki - 1))
                gh = [ps.tile([128, 32], f32, tag="ps", name=f"gh{g}") for g in range(3)]
                for k in range(8):
                    for g in range(3):
                        nc.tensor.matmul(gh[g][:], wh[:, k, g * 128:(g + 1) * 128],
                                         rhs_h[:, k, :],
                                         start=(k == 0), stop=(k == 7))
                # walrus: TensorTensor may read at most one operand from PSUM —
                # stage the gi gates through SBUF first.
                gis = []
                for g in range(3):
                    gsb = work.tile([128, 32], f32, tag=f"gis{g}")
                    nc.scalar.copy(gsb[:], gi[g][:])
                    gis.append(gsb)
                rt = work.tile([128, 32], f32, tag="rt")
                nc.vector.tensor_add(rt[:], gis[0][:], gh[0][:])
                rs = work.tile([128, 32], f32, tag="rs")
                nc.scalar.activation(rs[:], rt[:], Act.Sigmoid, bias=bg[:, bo:bo + 1])
                zt = work.tile([128, 32], f32, tag="zt")
                nc.vector.tensor_add(zt[:], gis[1][:], gh[1][:])
                zs = work.tile([128, 32], f32, tag="zs")
                nc.scalar.activation(zs[:], zt[:], Act.Sigmoid, bias=bg[:, bo + 1:bo + 2])
                hn = work.tile([128, 32], f32, tag="hn")
                nc.vector.tensor_scalar_add(hn[:], gh[2][:], bg[:, bo + 3:bo + 4])
                rn = work.tile([128, 32], f32, tag="rn")
                nc.vector.tensor_mul(rn[:], rs[:], hn[:])
                ns = work.tile([128, 32], f32, tag="ns")
                nc.vector.tensor_add(ns[:], gis[2][:], rn[:])
                nt = work.tile([128, 32], f32, tag="nt")
                nc.scalar.activation(nt[:], ns[:], Act.Tanh, bias=bg[:, bo + 2:bo + 3])
                dt_ = work.tile([128, 32], f32, tag="dt_")
                nc.vector.tensor_sub(dt_[:], hsl[:], nt[:])
                zd = work.tile([128, 32], f32, tag="zd")
                nc.vector.tensor_mul(zd[:], zs[:], dt_[:])
                hnew = res.tile([128, 32], f32, tag=htag)
                nc.vector.tensor_add(hnew[:], nt[:], zd[:])
                return hnew

            h0new = gru_layer(wih0T, whh0T, 16, rit, h0pt, h0sl, 0, "h0new")
            nc.sync.dma_start(h0T_o.ap(), h0new[:])
            h0bi = dram.tile([128, 32], f32, tag="h0bi")
            nc.sync.dma_start(h0bi[:], h0new[:])
            h0bo = dram.tile([1024, 32], f32, tag="h0bo")
            nc.gpsimd.collective_compute(
                "AllGather", Alu.bypass, replica_groups=GROUPS,
                ins=[h0bi.opt()], outs=[h0bo.opt()])
            h0all = res.tile([128, 8, 32], f32, tag="h0all")
            nc.sync.dma_start(h0all[:], h0bo.rearrange("(k p) b -> p k b", p=128))

            h1new = gru_layer(wih1T, whh1T, 8, h0all, h1pt, h1sl, 4, "h1new")
            nc.sync.dma_start(h1T_o.ap(), h1new[:])
            # b-major bounce for the second AllGather: h1new.T = [32, 128]
            tp0 = ps.tile([32, 128], f32, tag="ps")
            nc.tensor.transpose(tp0[:], h1new[:], eyet[:])
            h1bm = work.tile([32, 128], f32, tag="h1bm")
            nc.scalar.copy(h1bm[:], tp0[:])
            h1bi = dram.tile([32, 128], f32, tag="h1bi")
            nc.sync.dma_start(h1bi[:], h1bm[:])
            h1bo = dram.tile([256, 128], f32, tag="h1bo")
            nc.gpsimd.collective_compute(
                "AllGather", Alu.bypass, replica_groups=GROUPS,
                ins=[h1bi.opt()], outs=[h1bo.opt()])
            # h1 natural rows for all 32 b: [32, 8, 128]
            h1nat = res.tile([32, 8, 128], f32, tag="h1nat")
            nc.sync.dma_start(h1nat[:], h1bo.rearrange("(r b) p -> b r p", b=32))

            # h1T chunks [128, 32] (for the logits lhsT) + local-b h1 columns
            h1Tc = res.tile([128, 8, 32], f32, tag="h1Tc")
            h1loc = res.tile([128, 8, 4], f32, tag="h1loc")
            for k in range(8):
                tpk = ps.tile([128, 32], f32, tag="ps")
                nc.tensor.transpose(tpk[:], h1nat[:, k, :], eyet[:32, :32])
                nc.scalar.copy(h1Tc[:, k, :], tpk[:])
                lpk = ps.tile([128, 4], f32, tag="ps")
                nc.tensor.matmul(lpk[:], h1nat[:, k, :], bselt[:], start=True, stop=True)
                nc.scalar.copy(h1loc[:, k, :], lpk[:])

            # ---------------- q vectors (local 4 b) ----------------
            # qT[k, j] = sum_h W_attn[h, k] * h1_local[j, h]
            qps = [ps.tile([128, 4], f32, tag="ps", name=f"qps{k}") for k in range(8)]
            for hc in range(8):
                wat = pw2.tile([128, 1024], f32, tag="wat")
                nc.sync.dma_start(wat[:], wattn_d.ap()[hc * 128:(hc + 1) * 128, :])
                for kb in range(8):
                    nc.tensor.matmul(qps[kb][:], wat[:, kb * 128:(kb + 1) * 128],
                                     h1loc[:, hc, :],
                                     start=(hc == 0), stop=(hc == 7))
            qrhs = res.tile([128, 8, 4], f32, tag="qrhs")
            for kb in range(8):
                nc.scalar.copy(qrhs[:, kb, :], qps[kb][:])

            # ---------------- attention (per local b) ----------------
            attnT = res.tile([128, SCn, 4], f32, tag="attnT")
            if tiled:
                # 4 batch rows in 4 PE column groups, executed concurrently.
                ctxrow = res.tile([128, 1024], f32, tag="ctxrow")
                arow = res.tile([128, S_], f32, tag="arow")
                sps = [ps.tile([128, 512], f32, tag="ps", name=f"sps{k}")
                       for k in range(SBn)]
                ET_W = min(1024, S_)
                NS2 = ET_W // 512
                for kb in range(8):
                    for sh in range(S_ // ET_W):
                        ets = []
                        for j in range(4):
                            et = penc.tile([128, ET_W], f32, tag="et", name=f"et{j}")
                            nc.sync.dma_start(
                                et[:], encT_d.ap()[j, kb * 128:(kb + 1) * 128,
                                                   sh * ET_W:(sh + 1) * ET_W])
                            ets.append(et)
                        for s2 in range(NS2):
                            sbg = sh * NS2 + s2
                            for j in range(4):
                                nc.tensor.matmul(
                                    sps[sbg][32 * j:32 * j + 1, :],
                                    qrhs[:, kb, j:j + 1],
                                    ets[j][:, s2 * 512:(s2 + 1) * 512],
                                    start=(kb == 0), stop=(kb == 7),
                                    tile_position=(0, 32 * j))
                smax = work.tile([128, SBn], f32, tag="smax")
                negm = work.tile([128, 1], f32, tag="negm")
                z4 = work.tile([128, SBn], f32, tag="z4")
                rzj = work.tile([128, 1], f32, tag="rzj")
                for j in range(4):
                    p = slice(32 * j, 32 * j + 1)
                    for sb in range(SBn):
                        nc.vector.tensor_reduce(smax[p, sb:sb + 1], sps[sb][p, :],
                                                Ax.X, Alu.max)
                    nc.vector.tensor_reduce(negm[p, :], smax[p, :], Ax.X, Alu.max)
                    nc.vector.tensor_scalar_mul(negm[p, :], negm[p, :], -1.0)
                    for sb in range(SBn):
                        nc.scalar.activation(arow[p, sb * 512:(sb + 1) * 512],
                                             sps[sb][p, :], Act.Exp,
                                             bias=negm[p, :], scale=1.0,
                                             accum_out=z4[p, sb:sb + 1])
                    nc.vector.tensor_reduce(rzj[p, :], z4[p, :], Ax.X, Alu.add)
                    nc.vector.reciprocal(rzj[p, :], rzj[p, :])
                    nc.vector.tensor_scalar_mul(arow[p, :], arow[p, :], rzj[p, :])
                    nc.sync.dma_start(attn_o.ap()[j:j + 1, :], arow[p, :])
                    # transpose attn row via outer product in row-group 32j
                    tps = ps.tile([128, SCn], f32, tag="ps", name=f"tps{j}")
                    for sc in range(SCn):
                        nc.tensor.matmul(tps[:, sc:sc + 1],
                                         arow[p, sc * 128:(sc + 1) * 128],
                                         ones128[p, :], start=True, stop=True,
                                         tile_position=(32 * j, 0))
                    nc.scalar.copy(attnT[:, :, j], tps[:])
                cps = [ps.tile([128, 512], f32, tag="ps", name=f"cps{k}")
                       for k in range(2)]
                for sc in range(SCn):
                    ens = []
                    for j in range(4):
                        en = pencN.tile([128, 1024], f32, tag="en", name=f"en{j}")
                        nc.sync.dma_start(
                            en[:], encN_d.ap()[j, sc * 128:(sc + 1) * 128, :])
                        ens.append(en)
                    for hb in range(2):
                        for j in range(4):
                            nc.tensor.matmul(cps[hb][32 * j:32 * j + 1, :],
                                             attnT[:, sc, j:j + 1],
                                             ens[j][:, hb * 512:(hb + 1) * 512],
                                             start=(sc == 0), stop=(sc == SCn - 1),
                                             tile_position=(0, 32 * j))
                for j in range(4):
                    p = slice(32 * j, 32 * j + 1)
                    for hb in range(2):
                        nc.scalar.copy(ctxrow[p, hb * 512:(hb + 1) * 512],
                                       cps[hb][p, :])
                    nc.sync.dma_start(ctx_o.ap()[j:j + 1, :], ctxrow[p, :])
                ctxbi = dram.tile([4, 1024], f32, tag="ctxbi")
                for j in range(4):
                    nc.sync.dma_start(ctxbi[j:j + 1, :], ctxrow[32 * j:32 * j + 1, :])
            else:
                ctxrow = res.tile([1, 4, 1024], f32, tag="ctxrow")
                for j in range(4):
                    sps = [ps.tile([1, 512], f32, tag="ps", name=f"sps{k}") for k in range(SBn)]
                    for kb in range(8):
                        et = penc.tile([128, S_], f32, tag="et")
                        nc.sync.dma_start(et[:], encT_d.ap()[j, kb * 128:(kb + 1) * 128, :])
                        for sb in range(SBn):
                            nc.tensor.matmul(sps[sb][:], qrhs[:, kb, j:j + 1],
                                             et[:, sb * 512:(sb + 1) * 512],
                                             start=(kb == 0), stop=(kb == 7))
                    # softmax over s (exact, with global max)
                    m4 = work.tile([1, SBn], f32, tag="m4")
                    for sb in range(SBn):
                        nc.vector.tensor_reduce(m4[:, sb:sb + 1], sps[sb][:], Ax.X, Alu.max)
                    negm = work.tile([1, 1], f32, tag="negm")
                    nc.vector.tensor_reduce(negm[:], m4[:], Ax.X, Alu.max)
                    nc.vector.tensor_scalar_mul(negm[:], negm[:], -1.0)
                    arow = attnp.tile([1, S_], f32, tag="arow")
                    z4 = work.tile([1, SBn], f32, tag="z4")
                    for sb in range(SBn):
                        nc.scalar.activation(arow[:, sb * 512:(sb + 1) * 512], sps[sb][:],
                                             Act.Exp, bias=negm[:], scale=1.0,
                                             accum_out=z4[:, sb:sb + 1])
                    rzj = work.tile([1, 1], f32, tag="rzj")
                    nc.vector.tensor_reduce(rzj[:], z4[:], Ax.X, Alu.add)
                    nc.vector.reciprocal(rzj[:], rzj[:])
                    nc.vector.tensor_scalar_mul(arow[:], arow[:], rzj[:])
                    nc.sync.dma_start(attn_o.ap()[j:j + 1, :], arow[:])
                    # transpose attn row into [s-part, 1] chunks via outer product
                    tps = ps.tile([128, SCn], f32, tag="ps")
                    for sc in range(SCn):
                        nc.tensor.matmul(tps[:, sc:sc + 1], arow[:, sc * 128:(sc + 1) * 128],
                                         ones1[:], start=True, stop=True)
                    nc.scalar.copy(attnT[:, :, j], tps[:])
                    # context[j] = attn[j] @ encN[j]
                    cps = [ps.tile([1, 512], f32, tag="ps", name=f"cps{k}") for k in range(2)]
                    for sc in range(SCn):
                        en = pencN.tile([128, 1024], f32, tag="en")
                        nc.sync.dma_start(en[:], encN_d.ap()[j, sc * 128:(sc + 1) * 128, :])
                        for hb in range(2):
                            nc.tensor.matmul(cps[hb][:], attnT[:, sc, j:j + 1],
                                             en[:, hb * 512:(hb + 1) * 512],
                                             start=(sc == 0), stop=(sc == SCn - 1))
                    for hb in range(2):
                        nc.scalar.copy(ctxrow[0:1, j, hb * 512:(hb + 1) * 512], cps[hb][:])
                    nc.sync.dma_start(ctx_o.ap()[j:j + 1, :], ctxrow[0:1, j, :])
                ctxbi = dram.tile([4, 1024], f32, tag="ctxbi")
                for j in range(4):
                    nc.sync.dma_start(ctxbi[j:j + 1, :], ctxrow[0:1, j, :])

            # ---------------- gather context across cores ----------------
            ctxbo = dram.tile([32, 1024], f32, tag="ctxbo")
            nc.gpsimd.collective_compute(
                "AllGather", Alu.bypass, replica_groups=GROUPS,
                ins=[ctxbi.opt()], outs=[ctxbo.opt()])
            ctxnat = res.tile([32, 1024], f32, tag="ctxnat")
            nc.sync.dma_start(ctxnat[:], ctxbo[:])
            ctxTc = res.tile([128, 8, 32], f32, tag="ctxTc")
            for k in range(8):
                tpc = ps.tile([128, 32], f32, tag="ps")
                nc.tensor.transpose(tpc[:], ctxnat[:, k * 128:(k + 1) * 128],
                                    eyet[:32, :32])
                nc.scalar.copy(ctxTc[:, k, :], tpc[:])

            # ---------------- vocab-sharded logits + softmax ----------------
            expv = res.tile([32, VPC_], f32, tag="expv")
            zparts = work.tile([32, len(VB)], f32, tag="zparts")
            lps = [ps.tile([32, 512], f32, tag="ps", name=f"lps{k}") for k in range(len(VB))]
            for kc in range(16):
                wo = pbig.tile([128, VPC_], f32, tag="big")
                nc.sync.dma_start(wo[:], woutT_d.ap()[kc * 128:(kc + 1) * 128, :])
                lhs = h1Tc[:, kc, :] if kc < 8 else ctxTc[:, kc - 8, :]
                for vi, (vo, vw) in enumerate(VB):
                    nc.tensor.matmul(lps[vi][:, :vw], lhs, wo[:, vo:vo + vw],
                                     start=(kc == 0), stop=False)
            for vi, (vo, vw) in enumerate(VB):
                nc.tensor.matmul(lps[vi][:, :vw], ones32[:], boutt[:, vo:vo + vw],
                                 start=False, stop=True)
                nc.scalar.activation(expv[:, vo:vo + vw], lps[vi][:, :vw], Act.Exp,
                                     bias=0.0, scale=1.0,
                                     accum_out=zparts[:, vi:vi + 1])
            zbi = dram.tile([32, 1], f32, tag="zbi")
            zloc = work.tile([32, 1], f32, tag="zloc")
            nc.vector.tensor_reduce(zloc[:], zparts[:], Ax.X, Alu.add)
            nc.sync.dma_start(zbi[:], zloc[:])
            zbo = dram.tile([32, 1], f32, tag="zbo")
            nc.gpsimd.collective_compute(
                "AllReduce", Alu.add, replica_groups=GROUPS,
                ins=[zbi.opt()], outs=[zbo.opt()])
            rz = work.tile([32, 1], f32, tag="rz")
            nc.sync.dma_start(rz[:], zbo[:])
            nc.vector.reciprocal(rz[:], rz[:])
            nc.vector.tensor_scalar_mul(expv[:], expv[:], rz[:])
            nc.sync.dma_start(probs_o.ap(), expv[:])

    nc.compile()
    return nc


def host_prep(inputs, S_=S, V_=V):
    """Build the 8 per-core input maps from the full problem inputs."""
    VPC_ = V_ // NCORES
    ids = np.asarray(inputs["input_ids"])
    emb = np.asarray(inputs["emb"], np.float32)
    lc = np.asarray(inputs["last_context"], np.float32)
    hid = np.asarray(inputs["hidden"], np.float32)
    enc = np.ascontiguousarray(np.asarray(inputs["encoder_outputs"], np.float32))
    W_attn = np.ascontiguousarray(np.asarray(inputs["W_attn"], np.float32))
    W_out = np.asarray(inputs["W_out"], np.float32)
    b_out = np.asarray(inputs["b_out"], np.float32)

    x = emb[ids]                                        # [32, H]
    rnn_in = np.concatenate([x, lc[0]], axis=1)         # [32, 2H]
    rnn_inT = np.ascontiguousarray(rnn_in.T)            # [2H, 32]
    h0T = np.ascontiguousarray(hid[0].T)                # [H, 32]
    h1T = np.ascontiguousarray(hid[1].T)
    woutT_full = np.ascontiguousarray(W_out.T)          # [2H, V]

    def gate_cols(WT, c):
        Jc = slice(c * HPC, (c + 1) * HPC)
        return np.ascontiguousarray(
            np.concatenate([WT[:, g * H:(g + 1) * H][:, Jc] for g in range(3)], axis=1))

    wih0T = np.ascontiguousarray(np.asarray(inputs["Wih0"], np.float32).T)
    whh0T = np.ascontiguousarray(np.asarray(inputs["Whh0"], np.float32).T)
    wih1T = np.ascontiguousarray(np.asarray(inputs["Wih1"], np.float32).T)
    whh1T = np.ascontiguousarray(np.asarray(inputs["Whh1"], np.float32).T)
    bih0 = np.asarray(inputs["bih0"], np.float32)
    bhh0 = np.asarray(inputs["bhh0"], np.float32)
    bih1 = np.asarray(inputs["bih1"], np.float32)
    bhh1 = np.asarray(inputs["bhh1"], np.float32)
    eye = np.eye(128, dtype=np.float32)

    in_maps = []
    for c in range(NCORES):
        Jc = slice(c * HPC, (c + 1) * HPC)
        bg = np.stack([
            (bih0[0 * H:1 * H] + bhh0[0 * H:1 * H])[Jc],
            (bih0[1 * H:2 * H] + bhh0[1 * H:2 * H])[Jc],
            bih0[2 * H:3 * H][Jc],
            bhh0[2 * H:3 * H][Jc],
            (bih1[0 * H:1 * H] + bhh1[0 * H:1 * H])[Jc],
            (bih1[1 * H:2 * H] + bhh1[1 * H:2 * H])[Jc],
            bih1[2 * H:3 * H][Jc],
            bhh1[2 * H:3 * H][Jc],
        ], axis=1).astype(np.float32)                    # [128, 8]
        bsel = np.zeros((32, 4), np.float32)
        for jj in range(BPC):
            bsel[c * BPC + jj, jj] = 1.0
        bs = slice(c * BPC, (c + 1) * BPC)
        encN_c = np.ascontiguousarray(enc[bs, :S_, :])
        encT_c = np.ascontiguousarray(encN_c.transpose(0, 2, 1))
        in_maps.append({
            "rnn_inT": rnn_inT,
            "h0prevT": h0T, "h1prevT": h1T,
            "h0sl": np.ascontiguousarray(h0T[Jc]),
            "h1sl": np.ascontiguousarray(h1T[Jc]),
            "wih0T": gate_cols(wih0T, c), "whh0T": gate_cols(whh0T, c),
            "wih1T": gate_cols(wih1T, c), "whh1T": gate_cols(whh1T, c),
            "bgru": bg, "wattn": W_attn, "bsel": bsel, "eye": eye,
            "encT": encT_c, "encN": encN_c,
            "woutT": np.ascontiguousarray(woutT_full[:, c * VPC_:(c + 1) * VPC_]),
            "bout": np.ascontiguousarray(b_out[None, c * VPC_:(c + 1) * VPC_]),
        })
    return in_maps


def assemble(results, S_=S, V_=V):
    VPC_ = V_ // NCORES
    output = np.concatenate([results[c]["probs_o"] for c in range(NCORES)], axis=1)
    context = np.concatenate([results[c]["ctx_o"] for c in range(NCORES)], axis=0)
    attn = np.concatenate([results[c]["attn_o"] for c in range(NCORES)], axis=0)
    h0 = np.concatenate([results[c]["h0T_o"] for c in range(NCORES)], axis=0).T
    h1 = np.concatenate([results[c]["h1T_o"] for c in range(NCORES)], axis=0).T
    new_hidden = np.stack([h0, h1]).astype(np.float32)
    return (output.astype(np.float32),
            np.ascontiguousarray(context[None]).astype(np.float32),
            new_hidden,
            np.ascontiguousarray(attn[:, None, :]).astype(np.float32))


def kernel(**inputs):
    from concourse.bass_utils import run_bass_kernel_spmd
    if "nc" not in _cache:
        _cache["nc"] = build()
    in_maps = host_prep(inputs)
    res = run_bass_kernel_spmd(_cache["nc"], in_maps, core_ids=list(range(NCORES)))
    _cache["last_res"] = res
    return assemble(res.results)
